# revision 4
# baseline (speedup 1.0000x reference)
"""MultiHeadEMA on 8 Trainium2 NeuronCores.

Strategy
--------
Channel-sharded: embed_dim=1024 -> 8 slices of 128 channels (= SBUF
partitions), one per core. The reference's FFT conv is exactly an order-2 IIR
    y_n[l] = q_n y_n[l-1] + x[l],   out = silu(c0 y0 + c1 y1 + omega x)
computed with `tensor_tensor_scan` on the vector engine.

The DVE scan runs at ~2.2 cyc/elem, so the recurrence is decimated by 4:
    Y_n[j] = y_n[4j] satisfies  Y_n[j] = q_n^4 Y_n[j-1] + u_n[j]
    u_n[j] = x[4j] + q_n x[4j-1] + q_n^2 x[4j-2] + q_n^3 x[4j-3]
u_n is built by accumulating diagonal matmuls (tensor engine, bf16) into
PSUM from contiguous phase blocks of x (deinterleaved and pre-shifted on the
host). The scan reads u straight from PSUM at 1/4 length. Phase outputs
    pre_r = q0^r Y'_0 + q1^r Y'_1 + sum_s a_s x[4j+s]   (Y' = c_n-folded)
expand into diagonal matmuls accumulated in per-phase PSUM tiles; Silu
(ScalarE) evacuates PSUM -> SBUF -> DMA out (phase-major, host re-interleaves).

Engine balance (v3, from perfetto): PE matmuls issue at ~216ns (N=512);
DVE costs: scan 2280ns, tensor_scalar 478ns, tensor_tensor 684ns,
scalar_tensor_tensor 1270ns (1x mode only!) per 1024-wide op. Balancing
PE vs DVE puts phase 0 (pre0 = Y'0 + Y'1 + w x0, no PSUM needed) on DVE
as ts_mult + 2 adds, and keeps everything else on PE: 40 MMs/batch vs
DVE ~6.4us/batch. The per-channel scalar coefficients (18 values/channel
derived from delta/alpha/beta/gamma/omega) are precomputed on the host --
like `eye`, they are O(D) prep, and removing the on-device sigmoid chain
takes the ACT-table load + 13 serialized small ops off the critical path.
"""

import numpy as np
import ml_dtypes

import concourse.bass as bass
import concourse.bacc as bacc
import concourse.tile as tile
from concourse import mybir
from concourse.bass_utils import run_bass_kernel_spmd

SEQ_LEN, BSZ, EMBED_DIM, NDIM = 4096, 4, 1024, 2
N_CORES = 8
D_PER = EMBED_DIM // N_CORES  # 128 channels/core = full SBUF partitions
SCALE = (1.0 / NDIM) ** 0.5
DEC = 4                   # decimation factor
J = SEQ_LEN // DEC        # decimated length 1024
CH = 512                  # matmul chunk (one fp32 PSUM bank)
NG = J // CH              # j-groups per slab (2)
F32 = mybir.dt.float32
BF16 = mybir.dt.bfloat16
AF = mybir.ActivationFunctionType
ALU = mybir.AluOpType

# x phase blocks: r = 0..3 -> x[4j+r]. Shifted u-operands x[4j-k] are read
# as contiguous offset-(-1) views of block (4-k); only STRIDED rhs is slow.
NBLK = 4

# coef2 column layout (all fp32, per channel):
#  0..7: c_n q_n^k  for k=0..3, n=0,1  (order k-major: [c0,c1, cq0,cq1, ...])
#  8,9:   q_n
# 10,11:  q_n^2
# 12,13:  q_n^3
# 14,15:  q_n^4
# 16: c0+c1+w   17: c0 q0 + c1 q1   18: c0 q0^2 + c1 q1^2   19: w
NCOEF = 20


def build_bass():
    nc = bacc.Bacc(name="multihead_ema")
    x = nc.dram_tensor("x", [D_PER, BSZ, NBLK, J], BF16, kind="ExternalInput")
    coef = nc.dram_tensor("coef", [D_PER, NCOEF], F32, kind="ExternalInput")
    eye = nc.dram_tensor("eye", [D_PER, D_PER], BF16, kind="ExternalInput")
    out = nc.dram_tensor("out", [D_PER, BSZ, DEC, J], BF16, kind="ExternalOutput")

    with tile.TileContext(nc) as tc:
        with (
            tc.tile_pool(name="const", bufs=1) as const,
            tc.tile_pool(name="xup", bufs=4) as xup,
            tc.tile_pool(name="yp", bufs=3) as yp,
            tc.tile_pool(name="ysp", bufs=3) as ysp,
            tc.tile_pool(name="op", bufs=4) as op,
            tc.tile_pool(name="psu", bufs=2, space="PSUM") as psu,
            tc.tile_pool(name="psc", bufs=2, space="PSUM") as psc,
        ):
            csb = const.tile([D_PER, NCOEF], F32)
            nc.sync.dma_start(out=csb[:, :], in_=coef[:, :])
            eyesb = const.tile([D_PER, D_PER], BF16)
            nc.sync.dma_start(out=eyesb[:, :], in_=eye[:, :])

            # prefetch x slabs (b0 first so u-matmuls can start ASAP)
            xus = []
            for b in range(BSZ):
                xu = xup.tile([D_PER, 4, J], BF16, tag="xu")
                nc.sync.dma_start(out=xu[:, :, :], in_=x[:, b, :, :])
                xus.append(xu)

            # --- bf16 diagonal weight matrices from host-prepared scalars
            _dn = [0]

            def diag(col):
                _dn[0] += 1
                t = const.tile([D_PER, D_PER], BF16, tag=f"diag{_dn[0]}")
                nc.vector.tensor_scalar_mul(out=t[:, :], in0=eyesb[:, :],
                                            scalar1=csb[:, col : col + 1])
                return t

            # u-stage weights: c_n q_n^k (c folded into u/Y')
            w_cy = [[diag(2 * k + n) for n in range(NDIM)] for k in range(4)]
            # phase Y weights: q_n^r for r=1..3
            w_q = [[diag(8 + 2 * (r - 1) + n) for n in range(NDIM)]
                   for r in (1, 2, 3)]
            w_cw = diag(16)    # x_pr self term, r>=1
            w_cqs = diag(17)
            w_cq2s = diag(18)

            q4b = [csb[:, 14 + n : 15 + n].to_broadcast([D_PER, J])
                   for n in range(NDIM)]

            for b in range(BSZ):
                xu = xus[b]

                # --- u_n in PSUM, Y'_n = scan(q_n^4, u_n)   (c_n folded in)
                Y = []
                for n in range(NDIM):
                    pu = psu.tile([D_PER, J], F32, tag="u")
                    for g in range(NG):
                        s = bass.ts(g, CH)
                        nc.tensor.matmul(pu[:, s], w_cy[0][n][:, :], xu[:, 0, s],
                                         start=True, stop=False)
                        for k in range(1, 4):  # + c_n q^k * x[4j-k]
                            if g == 0:
                                nc.tensor.matmul(
                                    pu[:, 1:CH], w_cy[k][n][:, :],
                                    xu[:, 4 - k, 0 : CH - 1],
                                    start=False, stop=(k == 3))
                            else:
                                nc.tensor.matmul(
                                    pu[:, s], w_cy[k][n][:, :],
                                    xu[:, 4 - k, g * CH - 1 : (g + 1) * CH - 1],
                                    start=False, stop=(k == 3))
                    yn = yp.tile([D_PER, J], BF16, tag=f"y{n}")
                    nc.vector.tensor_tensor_scan(
                        out=yn[:, :], data0=q4b[n], data1=pu[:, :],
                        initial=0.0, op0=ALU.mult, op1=ALU.add,
                    )
                    Y.append(yn)

                ob = op.tile([D_PER, DEC, J], BF16)

                # --- phase 0 on DVE: pre0 = w*x0 + Y'0 + Y'1 (cheap op forms)
                t0 = ysp.tile([D_PER, J], BF16, tag="t0")
                nc.vector.tensor_scalar_mul(out=t0[:, :], in0=xu[:, 0, :],
                                            scalar1=csb[:, 19:20])
                t1 = ysp.tile([D_PER, J], BF16, tag="t1")
                nc.vector.tensor_add(out=t1[:, :], in0=t0[:, :], in1=Y[0][:, :])
                pre0 = ysp.tile([D_PER, J], BF16, tag="pre0")
                nc.vector.tensor_add(out=pre0[:, :], in0=t1[:, :], in1=Y[1][:, :])
                nc.scalar.activation(out=ob[:, 0, :], in_=pre0[:, :], func=AF.Silu)
                nc.sync.dma_start(out=out[:, b, 0, :], in_=ob[:, 0, :])

                # --- phases 1..3 on PE: x-terms + Y-terms in PSUM, silu out
                for r in (1, 2, 3):
                    pt = psc.tile([D_PER, J], F32, tag="pt")
                    for g in range(NG):
                        s = bass.ts(g, CH)
                        # x terms first (no scan dependency)
                        xw = [(w_cw, r)]
                        if r == 2:
                            xw.append((w_cqs, 1))
                        elif r == 3:
                            xw.append((w_cqs, 2))
                            xw.append((w_cq2s, 1))
                        for i, (wt, rr) in enumerate(xw):
                            nc.tensor.matmul(pt[:, s], wt[:, :], xu[:, rr, s],
                                             start=(i == 0), stop=False)
                        nc.tensor.matmul(pt[:, s], w_q[r - 1][0][:, :], Y[0][:, s],
                                         start=False, stop=False)
                        nc.tensor.matmul(pt[:, s], w_q[r - 1][1][:, :], Y[1][:, s],
                                         start=False, stop=True)
                    nc.scalar.activation(out=ob[:, r, :], in_=pt[:, :],
                                         func=AF.Silu)
                    nc.sync.dma_start(out=out[:, b, r, :], in_=ob[:, r, :])

    nc.compile()
    return nc


_CACHE: dict = {}


def _get_nc():
    if "nc" not in _CACHE:
        _CACHE["nc"] = build_bass()
    return _CACHE["nc"]


def make_in_maps(inputs):
    x = np.asarray(inputs["x"], np.float32)
    delta = np.asarray(inputs["delta"], np.float64).reshape(EMBED_DIM, NDIM)
    alpha = np.asarray(inputs["alpha"], np.float64).reshape(EMBED_DIM, NDIM)
    beta = np.asarray(inputs["beta"], np.float64).reshape(EMBED_DIM, NDIM)
    gamma = np.asarray(inputs["gamma"], np.float64).reshape(EMBED_DIM, NDIM)
    omega = np.asarray(inputs["omega"], np.float64).reshape(EMBED_DIM, 1)

    p = 1.0 / (1.0 + np.exp(-delta))
    q = 1.0 - p / (1.0 + np.exp(-alpha))          # [D, 2]
    c = p * beta * gamma * SCALE                   # [D, 2]
    cols = []
    for k in range(4):
        cols.append(c * q**k)                      # c_n q_n^k  [D, 2]
    cols.extend([q, q**2, q**3, q**4])
    cols.append((c.sum(1) + omega[:, 0])[:, None])           # c0+c1+w
    cols.append((c * q).sum(1)[:, None])                     # cqs
    cols.append((c * q**2).sum(1)[:, None])                  # cq2s
    cols.append(omega)                                       # w
    coef_full = np.concatenate(cols, axis=1).astype(np.float32)
    assert coef_full.shape == (EMBED_DIM, NCOEF)

    eye = np.eye(D_PER, dtype=ml_dtypes.bfloat16)
    in_maps = []
    for c_ in range(N_CORES):
        sl = slice(c_ * D_PER, (c_ + 1) * D_PER)
        xc = x[:, :, sl].transpose(2, 1, 0).astype(ml_dtypes.bfloat16)  # [128,B,L]
        ph = xc.reshape(D_PER, BSZ, J, DEC).transpose(0, 1, 3, 2)  # [128,B,4,J]
        in_maps.append(
            {"x": np.ascontiguousarray(ph),
             "coef": np.ascontiguousarray(coef_full[sl]), "eye": eye}
        )
    return in_maps


def gather_out(results):
    out = np.empty((SEQ_LEN, BSZ, EMBED_DIM), np.float32)
    for c in range(N_CORES):
        # [128, B, 4, J] phase-major -> [l = 4j+r, b, d]
        arr = results[c]["out"].astype(np.float32)
        out[:, :, c * D_PER : (c + 1) * D_PER] = arr.transpose(3, 2, 1, 0).reshape(
            SEQ_LEN, BSZ, D_PER
        )
    return out


def _run(inputs, **kwargs):
    nc = _get_nc()
    in_maps = make_in_maps(inputs)
    res = run_bass_kernel_spmd(nc, in_maps, core_ids=list(range(N_CORES)), **kwargs)
    return gather_out(res.results), res


def kernel(**inputs) -> np.ndarray:
    out, _ = _run(inputs)
    return out


# revision 6
# speedup vs baseline: 1.0244x; 1.0244x over previous
"""MultiHeadEMA on 8 Trainium2 NeuronCores.

Strategy
--------
Channel-sharded: embed_dim=1024 -> 8 slices of 128 channels (= SBUF
partitions), one per core. The reference's FFT conv is exactly an order-2 IIR
    y_n[l] = q_n y_n[l-1] + x[l],   out = silu(c0 y0 + c1 y1 + omega x)
computed with `tensor_tensor_scan` on the vector engine.

The DVE scan runs at ~2.2 cyc/elem, so the recurrence is decimated by 4:
    Y_n[j] = y_n[4j] satisfies  Y_n[j] = q_n^4 Y_n[j-1] + u_n[j]
    u_n[j] = x[4j] + q_n x[4j-1] + q_n^2 x[4j-2] + q_n^3 x[4j-3]
u_n is built by accumulating diagonal matmuls (tensor engine, bf16) into
PSUM from contiguous phase blocks of x (deinterleaved and pre-shifted on the
host). The scan reads u straight from PSUM at 1/4 length. Phase outputs
    pre_r = q0^r Y'_0 + q1^r Y'_1 + sum_s a_s x[4j+s]   (Y' = c_n-folded)
expand into diagonal matmuls accumulated in per-phase PSUM tiles; Silu
(ScalarE) evacuates PSUM -> SBUF -> DMA out (phase-major, host re-interleaves).

Engine balance (v3, from perfetto): PE matmuls issue at ~216ns (N=512);
DVE costs: scan 2280ns, tensor_scalar 478ns, tensor_tensor 684ns,
scalar_tensor_tensor 1270ns (1x mode only!) per 1024-wide op. Balancing
PE vs DVE puts phase 0 (pre0 = Y'0 + Y'1 + w x0, no PSUM needed) on DVE
as ts_mult + 2 adds, and keeps everything else on PE: 40 MMs/batch vs
DVE ~6.4us/batch. The per-channel scalar coefficients (18 values/channel
derived from delta/alpha/beta/gamma/omega) are precomputed on the host --
like `eye`, they are O(D) prep, and removing the on-device sigmoid chain
takes the ACT-table load + 13 serialized small ops off the critical path.
"""

import numpy as np
import ml_dtypes

import concourse.bass as bass
import concourse.bacc as bacc
import concourse.tile as tile
from concourse import mybir
from concourse.bass_utils import run_bass_kernel_spmd

SEQ_LEN, BSZ, EMBED_DIM, NDIM = 4096, 4, 1024, 2
N_CORES = 8
D_PER = EMBED_DIM // N_CORES  # 128 channels/core = full SBUF partitions
SCALE = (1.0 / NDIM) ** 0.5
DEC = 4                   # decimation factor
J = SEQ_LEN // DEC        # decimated length 1024
CH = 512                  # matmul chunk (one fp32 PSUM bank)
NG = J // CH              # j-groups per slab (2)
F32 = mybir.dt.float32
BF16 = mybir.dt.bfloat16
AF = mybir.ActivationFunctionType
ALU = mybir.AluOpType

# x phase blocks: r = 0..3 -> x[4j+r]. Shifted u-operands x[4j-k] are read
# as contiguous offset-(-1) views of block (4-k); only STRIDED rhs is slow.
NBLK = 4

# coef2 column layout (all fp32, per channel):
#  0..7: c_n q_n^k  for k=0..3, n=0,1  (order k-major: [c0,c1, cq0,cq1, ...])
#  8,9:   q_n
# 10,11:  q_n^2
# 12,13:  q_n^3
# 14,15:  q_n^4
# 16: c0+c1+w   17: c0 q0 + c1 q1   18: c0 q0^2 + c1 q1^2   19: w
NCOEF = 20


def build_bass():
    nc = bacc.Bacc(name="multihead_ema")
    x = nc.dram_tensor("x", [D_PER, BSZ, NBLK, J], BF16, kind="ExternalInput")
    coef = nc.dram_tensor("coef", [D_PER, NCOEF], F32, kind="ExternalInput")
    eye = nc.dram_tensor("eye", [D_PER, D_PER], BF16, kind="ExternalInput")
    out = nc.dram_tensor("out", [D_PER, BSZ, DEC, J], BF16, kind="ExternalOutput")

    with tile.TileContext(nc) as tc:
        with (
            tc.tile_pool(name="const", bufs=1) as const,
            tc.tile_pool(name="xup", bufs=4) as xup,
            tc.tile_pool(name="yp", bufs=3) as yp,
            tc.tile_pool(name="ysp", bufs=3) as ysp,
            tc.tile_pool(name="op", bufs=4) as op,
            tc.tile_pool(name="psu", bufs=2, space="PSUM") as psu,
            tc.tile_pool(name="psc", bufs=2, space="PSUM") as psc,
        ):
            csb = const.tile([D_PER, NCOEF], F32)
            nc.sync.dma_start(out=csb[:, :], in_=coef[:, :])
            eyesb = const.tile([D_PER, D_PER], BF16)
            nc.sync.dma_start(out=eyesb[:, :], in_=eye[:, :])

            # prefetch x slabs (b0 first so u-matmuls can start ASAP);
            # alternate DMA queues so transfers overlap
            xus = []
            for b in range(BSZ):
                xu = xup.tile([D_PER, 4, J], BF16, tag="xu")
                eng = nc.sync if b % 2 == 0 else nc.gpsimd
                eng.dma_start(out=xu[:, :, :], in_=x[:, b, :, :])
                xus.append(xu)

            # --- bf16 diagonal weight matrices from host-prepared scalars
            _dn = [0]

            def diag(col):
                _dn[0] += 1
                t = const.tile([D_PER, D_PER], BF16, tag=f"diag{_dn[0]}")
                nc.vector.tensor_scalar_mul(out=t[:, :], in0=eyesb[:, :],
                                            scalar1=csb[:, col : col + 1])
                return t

            # u-stage weights: c_n q_n^k (c folded into u/Y')
            w_cy = [[diag(2 * k + n) for n in range(NDIM)] for k in range(4)]
            # phase Y weights: q_n^r for r=1..3
            w_q = [[diag(8 + 2 * (r - 1) + n) for n in range(NDIM)]
                   for r in (1, 2, 3)]
            w_cw = diag(16)    # x_pr self term, r>=1
            w_cqs = diag(17)
            w_cq2s = diag(18)

            q4b = [csb[:, 14 + n : 15 + n].to_broadcast([D_PER, J])
                   for n in range(NDIM)]

            def emit_u(b):
                """u_n matmuls into PSUM + DVE scans -> Y'_n (c_n folded)."""
                xu = xus[b]
                # pre0's w*x0 only needs xu: pack it early on the DVE queue
                t0 = ysp.tile([D_PER, J], BF16, tag="t0")
                nc.vector.tensor_scalar_mul(out=t0[:, :], in0=xu[:, 0, :],
                                            scalar1=csb[:, 19:20])
                Y = []
                for n in range(NDIM):
                    pu = psu.tile([D_PER, J], F32, tag="u")
                    for g in range(NG):
                        s = bass.ts(g, CH)
                        nc.tensor.matmul(pu[:, s], w_cy[0][n][:, :], xu[:, 0, s],
                                         start=True, stop=False)
                        for k in range(1, 4):  # + c_n q^k * x[4j-k]
                            if g == 0:
                                nc.tensor.matmul(
                                    pu[:, 1:CH], w_cy[k][n][:, :],
                                    xu[:, 4 - k, 0 : CH - 1],
                                    start=False, stop=(k == 3))
                            else:
                                nc.tensor.matmul(
                                    pu[:, s], w_cy[k][n][:, :],
                                    xu[:, 4 - k, g * CH - 1 : (g + 1) * CH - 1],
                                    start=False, stop=(k == 3))
                    yn = yp.tile([D_PER, J], BF16, tag=f"y{n}")
                    nc.vector.tensor_tensor_scan(
                        out=yn[:, :], data0=q4b[n], data1=pu[:, :],
                        initial=0.0, op0=ALU.mult, op1=ALU.add,
                    )
                    Y.append(yn)
                return t0, Y

            def emit_phases(b, t0, Y):
                xu = xus[b]
                ob = op.tile([D_PER, DEC, J], BF16)

                # --- phase 0 on DVE: pre0 = w*x0 + Y'0 + Y'1
                t1 = ysp.tile([D_PER, J], BF16, tag="t1")
                nc.vector.tensor_add(out=t1[:, :], in0=t0[:, :], in1=Y[0][:, :])
                pre0 = ysp.tile([D_PER, J], BF16, tag="pre0")
                nc.vector.tensor_add(out=pre0[:, :], in0=t1[:, :], in1=Y[1][:, :])
                nc.scalar.activation(out=ob[:, 0, :], in_=pre0[:, :], func=AF.Silu)
                nc.sync.dma_start(out=out[:, b, 0, :], in_=ob[:, 0, :])

                # --- phases 1..3 on PE: x-terms + Y-terms in PSUM, silu out
                for r in (1, 2, 3):
                    pt = psc.tile([D_PER, J], F32, tag="pt")
                    for g in range(NG):
                        s = bass.ts(g, CH)
                        # x terms first (no scan dependency)
                        xw = [(w_cw, r)]
                        if r == 2:
                            xw.append((w_cqs, 1))
                        elif r == 3:
                            xw.append((w_cqs, 2))
                            xw.append((w_cq2s, 1))
                        for i, (wt, rr) in enumerate(xw):
                            nc.tensor.matmul(pt[:, s], wt[:, :], xu[:, rr, s],
                                             start=(i == 0), stop=False)
                        nc.tensor.matmul(pt[:, s], w_q[r - 1][0][:, :], Y[0][:, s],
                                         start=False, stop=False)
                        nc.tensor.matmul(pt[:, s], w_q[r - 1][1][:, :], Y[1][:, s],
                                         start=False, stop=True)
                    nc.scalar.activation(out=ob[:, r, :], in_=pt[:, :],
                                         func=AF.Silu)
                    nc.sync.dma_start(out=out[:, b, r, :], in_=ob[:, r, :])

            # software pipeline: u(b+1) is queued on PE before phases(b) so
            # the PE never stalls waiting for batch b's scans
            pend = emit_u(0)
            for b in range(BSZ):
                nxt = emit_u(b + 1) if b + 1 < BSZ else None
                emit_phases(b, *pend)
                pend = nxt

    nc.compile()
    return nc


_CACHE: dict = {}


def _get_nc():
    if "nc" not in _CACHE:
        _CACHE["nc"] = build_bass()
    return _CACHE["nc"]


def make_in_maps(inputs):
    x = np.asarray(inputs["x"], np.float32)
    delta = np.asarray(inputs["delta"], np.float64).reshape(EMBED_DIM, NDIM)
    alpha = np.asarray(inputs["alpha"], np.float64).reshape(EMBED_DIM, NDIM)
    beta = np.asarray(inputs["beta"], np.float64).reshape(EMBED_DIM, NDIM)
    gamma = np.asarray(inputs["gamma"], np.float64).reshape(EMBED_DIM, NDIM)
    omega = np.asarray(inputs["omega"], np.float64).reshape(EMBED_DIM, 1)

    p = 1.0 / (1.0 + np.exp(-delta))
    q = 1.0 - p / (1.0 + np.exp(-alpha))          # [D, 2]
    c = p * beta * gamma * SCALE                   # [D, 2]
    cols = []
    for k in range(4):
        cols.append(c * q**k)                      # c_n q_n^k  [D, 2]
    cols.extend([q, q**2, q**3, q**4])
    cols.append((c.sum(1) + omega[:, 0])[:, None])           # c0+c1+w
    cols.append((c * q).sum(1)[:, None])                     # cqs
    cols.append((c * q**2).sum(1)[:, None])                  # cq2s
    cols.append(omega)                                       # w
    coef_full = np.concatenate(cols, axis=1).astype(np.float32)
    assert coef_full.shape == (EMBED_DIM, NCOEF)

    eye = np.eye(D_PER, dtype=ml_dtypes.bfloat16)
    in_maps = []
    for c_ in range(N_CORES):
        sl = slice(c_ * D_PER, (c_ + 1) * D_PER)
        xc = x[:, :, sl].transpose(2, 1, 0).astype(ml_dtypes.bfloat16)  # [128,B,L]
        ph = xc.reshape(D_PER, BSZ, J, DEC).transpose(0, 1, 3, 2)  # [128,B,4,J]
        in_maps.append(
            {"x": np.ascontiguousarray(ph),
             "coef": np.ascontiguousarray(coef_full[sl]), "eye": eye}
        )
    return in_maps


def gather_out(results):
    out = np.empty((SEQ_LEN, BSZ, EMBED_DIM), np.float32)
    for c in range(N_CORES):
        # [128, B, 4, J] phase-major -> [l = 4j+r, b, d]
        arr = results[c]["out"].astype(np.float32)
        out[:, :, c * D_PER : (c + 1) * D_PER] = arr.transpose(3, 2, 1, 0).reshape(
            SEQ_LEN, BSZ, D_PER
        )
    return out


def _run(inputs, **kwargs):
    nc = _get_nc()
    in_maps = make_in_maps(inputs)
    res = run_bass_kernel_spmd(nc, in_maps, core_ids=list(range(N_CORES)), **kwargs)
    return gather_out(res.results), res


def kernel(**inputs) -> np.ndarray:
    out, _ = _run(inputs)
    return out
